# revision 20
# baseline (speedup 1.0000x reference)
"""Trainium2 Bass kernel for nn_MultiHeadAttention (dense transformer prefill,
GQA 32q/8kv heads, RoPE, causal mask), tensor-parallel over heads across 8
NeuronCores with NO device collective: each core emits a full-shape partial
output (its 4 q heads contracted against the matching 512 columns of wo) and
the host sums the 8 partials during the gather step.

v4 over v3 (820us -> target ~700us):
  - Per-batch two-phase schedule: all 4 tiles' projections emitted as one
    dense GEMM block, then an attention sweep where the previous tile's
    output-projection MM groups are interleaved as PE filler work between
    attention chunks (pending queue carried across tiles AND batches), so
    the PE never idles while ACT grinds the exp chain.
  - All PSUM evictions moved off ACT (was 48% busy inside the exp critical
    chain) onto DVE; softmax denominator accumulation moved to the idle
    GpSimd engine; ACT now does exp only.
  - mg = o * rec multiplies straight out of PSUM on DVE (no o_sb ACT copy);
    per-head softmax tail (den_bf -> ones-matmul -> recip -> mul) deferred
    into the next head's chunk stream so the PE ones-matmul never stalls.
  - Single merged PSUM pool (4 banks) for proj streams + scores + V
    transposes; 2 banks for attention out; 2 for outproj/ones broadcast.
  - Tile-0 startup: K/V/Q0/Q1 streams quarter-interleaved with the x and
    weight quarter DMAs so the PE ramps at DMA delivery rate.
"""

import math
from collections import deque

import numpy as np
import ml_dtypes

import concourse.bass as bass
import concourse.tile as tile
from concourse import bacc, mybir
from concourse import masks
from concourse.bass_utils import run_bass_kernel_spmd

BF16 = ml_dtypes.bfloat16

# ---- problem constants ----
B = 2
S = 2048
D = 4096
N_HEADS = 32
N_KV = 8
HD = 128
NCORES = 8
T = B * S                      # 4096 flattened tokens
HLOC = N_HEADS // NCORES       # 4 local q heads
E = HLOC * HD                  # 512 local q dim
DCH = D // 128                 # 32 contraction chunks
KC = S // 128                  # 16 k-chunks per sequence
KG = KC // 2                   # 8 k-groups (2 chunks) per sequence
QT = S // 512                  # 4 q-tiles per sequence
SCALE = 1.0 / math.sqrt(HD)

SKIP, PLAIN, MIXED_CAUSAL, MIXED_DATA = 0, 1, 2, 3


def _classify_mask(mask):
    """Classify (k-group, q-tile) tiles of mask^T. Returns (plan, mtiles).

    plan[qt] = list of (kg, mode, mtile_idx); mtiles: (n, 128, 2, 512) f32."""
    m = np.asarray(mask, np.float32).reshape(S, S)  # [q, k]
    mt = np.ascontiguousarray(m.T)  # [k, q]
    kk = np.arange(S)
    qq = np.arange(S)
    plan = []
    uniq = {}
    mtiles = []
    for qt in range(QT):
        row = []
        qs = slice(qt * 512, (qt + 1) * 512)
        for kg in range(KG):
            ks = slice(kg * 256, (kg + 1) * 256)
            sub = mt[ks, qs]
            if np.all(sub <= -1e8):
                continue  # fully masked -> skip the whole k-group
            if np.all(sub == 0.0):
                row.append((kg, PLAIN, -1))
                continue
            # mixed: exact-causal check (0 where q>=k, <=-1e8 where q<k)
            causal = qq[None, qs] >= kk[ks, None]
            if np.all((sub == 0.0) == causal) and np.all(sub[~causal] <= -1e8):
                row.append((kg, MIXED_CAUSAL, -1))
            else:
                key = sub.tobytes()
                if key not in uniq:
                    uniq[key] = len(mtiles)
                    mtiles.append(sub.reshape(2, 128, 512).transpose(1, 0, 2))
                row.append((kg, MIXED_DATA, uniq[key]))
        plan.append(row)
    if mtiles:
        mtiles = np.ascontiguousarray(np.stack(mtiles), np.float32)
    else:
        mtiles = np.zeros((1, 128, 2, 512), np.float32)
    return plan, mtiles


def _pair_split_perm(nheads):
    """Row permutation putting each head's even components first, odds second."""
    idx = []
    for h in range(nheads):
        base = h * HD
        idx.extend(base + np.arange(0, HD, 2))
        idx.extend(base + np.arange(1, HD, 2))
    return np.asarray(idx)


def _build(plan, n_mtiles):
    nc = bacc.Bacc("TRN2", target_bir_lowering=False, debug=False,
                   num_devices=NCORES)
    f32, bf = mybir.dt.float32, mybir.dt.bfloat16

    xt = nc.dram_tensor("xt", [D, T], bf, kind="ExternalInput").ap()
    wqt = nc.dram_tensor("wqt", [D, E], bf, kind="ExternalInput").ap()
    wkt = nc.dram_tensor("wkt", [D, HD], bf, kind="ExternalInput").ap()
    wvt = nc.dram_tensor("wvt", [D, HD], bf, kind="ExternalInput").ap()
    wol = nc.dram_tensor("wol", [E, D], bf, kind="ExternalInput").ap()
    csa = nc.dram_tensor("csa", [128, S], bf, kind="ExternalInput").ap()
    csb = nc.dram_tensor("csb", [128, S], bf, kind="ExternalInput").ap()
    mtl = nc.dram_tensor("mtl", [n_mtiles, 128, 2, 512], f32,
                         kind="ExternalInput").ap()
    out = nc.dram_tensor("out", [T, D], bf, kind="ExternalOutput").ap()

    with tile.TileContext(nc) as tc:
        _emit(nc, tc, plan, xt, wqt, wkt, wvt, wol, csa, csb, mtl, out)
    nc.compile()
    return nc


def _emit(nc, tc, plan, xt, wqt, wkt, wvt, wol, csa, csb, mtl, out):
    f32, bf = mybir.dt.float32, mybir.dt.bfloat16
    Exp = mybir.ActivationFunctionType.Exp

    xt_r = xt.rearrange("(c p) t -> p c t", p=128)
    out_r = out.rearrange("(tb p) d -> p tb d", p=128)
    wqt_r = wqt.rearrange("(c p) e -> p c e", p=128)
    wkt_r = wkt.rearrange("(c p) e -> p c e", p=128)
    wvt_r = wvt.rearrange("(c p) e -> p c e", p=128)
    wol_r = wol.rearrange("(h p) d -> p h d", p=128)

    QC = DCH // 4   # 8 contraction chunks per x quarter

    from contextlib import ExitStack
    with ExitStack() as stk:
        singles = stk.enter_context(tc.tile_pool(name="singles", bufs=1))
        xts = stk.enter_context(tc.tile_pool(name="xts", bufs=6))
        kvp = stk.enter_context(tc.tile_pool(name="kv", bufs=1))
        qcp = stk.enter_context(tc.tile_pool(name="qc", bufs=1))
        vtsp = stk.enter_context(tc.tile_pool(name="vts", bufs=2))
        ptp = stk.enter_context(tc.tile_pool(name="ptp", bufs=8))
        rp = stk.enter_context(tc.tile_pool(name="rp", bufs=2))
        dnp = stk.enter_context(tc.tile_pool(name="dna", bufs=3))
        recp = stk.enter_context(tc.tile_pool(name="recp", bufs=2))
        mgp = stk.enter_context(tc.tile_pool(name="mgp", bufs=2))
        mkp = stk.enter_context(tc.tile_pool(name="mkp", bufs=2))
        outp = stk.enter_context(tc.tile_pool(name="outp", bufs=4))
        # PSUM: psA (proj streams + scores + V-transposes) 4 banks,
        # ps_o (attention out) 2, ps_c (outproj groups + ones bcast) 2.
        psA = stk.enter_context(tc.tile_pool(name="psA", bufs=4, space="PSUM"))
        ps_o = stk.enter_context(
            tc.tile_pool(name="ps_o", bufs=2, space="PSUM"))
        ps_c = stk.enter_context(
            tc.tile_pool(name="ps_c", bufs=2, space="PSUM"))

        # --- resident weights / tables (startup-ordered DMAs) ---
        wq_sb = singles.tile([128, DCH, E], bf)
        wk_sb = singles.tile([128, DCH, HD], bf)
        wv_sb = singles.tile([128, DCH, HD], bf)
        wol_sb = singles.tile([128, HLOC, D], bf)
        csa_sb = singles.tile([128, S], bf)
        csb_sb = singles.tile([128, S], bf)
        ones128 = singles.tile([128, 128], bf)
        nc.vector.memset(ones128, 1.0)
        ident = singles.tile([128, 128], bf)
        masks.make_identity(nc, ident)

        def load_weight_quarter(i4):
            nc.sync.dma_start(out=wk_sb[:, i4 * QC:(i4 + 1) * QC, :],
                              in_=wkt_r[:, i4 * QC:(i4 + 1) * QC, :])
            nc.sync.dma_start(out=wv_sb[:, i4 * QC:(i4 + 1) * QC, :],
                              in_=wvt_r[:, i4 * QC:(i4 + 1) * QC, :])
            nc.sync.dma_start(out=wq_sb[:, i4 * QC:(i4 + 1) * QC, :],
                              in_=wqt_r[:, i4 * QC:(i4 + 1) * QC, :])

        def load_rope_tables():
            nc.sync.dma_start(out=csa_sb, in_=csa)
            nc.sync.dma_start(out=csb_sb, in_=csb)

        def load_late_weights():
            nc.sync.dma_start(out=wol_sb, in_=wol_r)

        def apply_rope(ps_tile, dst, tloc, width):
            """dst = RoPE(ps_tile) using csa/csb tables at in-seq pos tloc."""
            ta = rp.tile([128, 512], bf, tag="ta")
            ta = ta[:, :width]
            nc.vector.tensor_mul(ta, ps_tile, csa_sb[:, tloc:tloc + width])
            tb = rp.tile([128, 512], bf, tag="tb")
            tb = tb[:, :width]
            nc.vector.tensor_mul(tb[0:64, :], ps_tile[64:128, :],
                                 csb_sb[0:64, tloc:tloc + width])
            nc.vector.tensor_mul(tb[64:128, :], ps_tile[0:64, :],
                                 csb_sb[64:128, tloc:tloc + width])
            nc.vector.tensor_add(dst, ta, tb)

        # deferred per-head softmax tails; emitted a couple of chunks into
        # the next head so the PE ones-matmul never stalls on the den chain
        deferred = []

        def emit_deferred():
            while deferred:
                deferred.pop()()

        # pending output-projection MM groups (PE filler work); carried
        # across tiles and batches
        pending = deque()

        def emit_filler():
            if pending:
                pending.popleft()()

        # V-transpose work deferred by one tile so the PE transposes never
        # wait on the vts DVE eviction
        pending_vt = [None]

        def emit_vt():
            if pending_vt[0] is not None:
                pending_vt[0]()
                pending_vt[0] = None

        for b in range(B):
            # ---------------- PROJ PHASE (4 q-tiles) ----------------
            k_t = kvp.tile([128, S], bf, tag="k")
            v_t = kvp.tile([128, KC, HD], bf, tag="v")
            q_all = []

            for tt in range(QT):
                t0 = b * S + tt * 512
                tloc = tt * 512
                xh = [xts.tile([128, QC, 512], bf, tag="x", name=f"xh{i}")
                      for i in range(4)]
                if b == 0 and tt == 0:
                    # startup: x slices on the Sync HWDGE queue, wq slices
                    # on the ACT HWDGE queue — two queues spin up and
                    # transfer in parallel; wk/wv (slow 256B-line DMAs)
                    # off the critical path (K/V streams run last)
                    for s in range(QC // 2):
                        sl = slice(2 * s, 2 * s + 2)
                        nc.sync.dma_start(out=xh[0][:, sl, :],
                                          in_=xt_r[:, sl, t0:t0 + 512])
                        nc.scalar.dma_start(out=wq_sb[:, sl, :],
                                            in_=wqt_r[:, sl, :])
                    load_rope_tables()
                    for i4 in range(1, 4):
                        for hs in range(2):
                            sl = slice(i4 * QC + hs * (QC // 2),
                                       i4 * QC + (hs + 1) * (QC // 2))
                            eng = nc.sync if hs == 0 else nc.scalar
                            eng.dma_start(out=xh[i4][:, hs * (QC // 2):
                                                     (hs + 1) * (QC // 2), :],
                                          in_=xt_r[:, sl, t0:t0 + 512])
                        nc.scalar.dma_start(
                            out=wq_sb[:, i4 * QC:(i4 + 1) * QC, :],
                            in_=wqt_r[:, i4 * QC:(i4 + 1) * QC, :])
                    nc.sync.dma_start(out=wk_sb, in_=wkt_r)
                    nc.sync.dma_start(out=wv_sb, in_=wvt_r)
                    load_late_weights()
                else:
                    # alternate x halves across the two HWDGE queues
                    for i4 in range(4):
                        for hs in range(2):
                            sl = slice(i4 * QC + hs * (QC // 2),
                                       i4 * QC + (hs + 1) * (QC // 2))
                            eng = nc.sync if hs == 0 else nc.scalar
                            eng.dma_start(out=xh[i4][:, hs * (QC // 2):
                                                     (hs + 1) * (QC // 2), :],
                                          in_=xt_r[:, sl, t0:t0 + 512])

                def xsl(ci):
                    return xh[ci // QC][:, ci % QC, :]

                q_cur = qcp.tile([128, HLOC, 512], bf, tag=f"qc{tt}",
                                 name=f"qc{tt}")
                q_all.append(q_cur)

                if b == 0 and tt == 0:
                    # quarter-major interleave of the 4 Q streams so the PE
                    # ramps at DMA delivery rate (wq has fast 1KB lines);
                    # K/V streams follow once wk/wv have landed
                    q_pss = [psA.tile([128, 512], f32, tag="a",
                                      name=f"q{h}_ps") for h in range(HLOC)]
                    for i4 in range(4):
                        for ci in range(i4 * QC, (i4 + 1) * QC):
                            st, sp = (ci == 0), (ci == DCH - 1)
                            for h in range(HLOC):
                                nc.tensor.matmul(
                                    q_pss[h], wq_sb[:, ci, h * HD:(h + 1) * HD],
                                    xsl(ci), start=st, stop=sp,
                                    skip_group_check=True)
                    for h in range(HLOC):
                        apply_rope(q_pss[h], q_cur[:, h, :], tloc, 512)
                    # K projection + RoPE
                    k_ps = psA.tile([128, 512], f32, tag="a", name="k_ps")
                    for ci in range(DCH):
                        nc.tensor.matmul(k_ps, wk_sb[:, ci, :], xsl(ci),
                                         start=(ci == 0), stop=(ci == DCH - 1))
                    apply_rope(k_ps, k_t[:, tloc:tloc + 512], tloc, 512)
                    # V^T projection
                    vt_ps = psA.tile([128, 512], f32, tag="a", name="vt_ps")
                    for ci in range(DCH):
                        nc.tensor.matmul(vt_ps, wv_sb[:, ci, :], xsl(ci),
                                         start=(ci == 0), stop=(ci == DCH - 1))
                    vts = vtsp.tile([128, 512], bf, tag="vts")
                    nc.vector.tensor_copy(vts, vt_ps)
                else:
                    # Q projections + RoPE
                    for h in range(HLOC):
                        q_ps = psA.tile([128, 512], f32, tag="a", name="q_ps")
                        for ci in range(DCH):
                            nc.tensor.matmul(
                                q_ps, wq_sb[:, ci, h * HD:(h + 1) * HD],
                                xsl(ci), start=(ci == 0), stop=(ci == DCH - 1))
                        apply_rope(q_ps, q_cur[:, h, :], tloc, 512)
                        if h == 0:
                            emit_vt()
                    # K projection + RoPE
                    k_ps = psA.tile([128, 512], f32, tag="a", name="k_ps")
                    for ci in range(DCH):
                        nc.tensor.matmul(k_ps, wk_sb[:, ci, :], xsl(ci),
                                         start=(ci == 0), stop=(ci == DCH - 1))
                    apply_rope(k_ps, k_t[:, tloc:tloc + 512], tloc, 512)
                    # V^T projection (e-major wide matmuls)
                    vt_ps = psA.tile([128, 512], f32, tag="a", name="vt_ps")
                    for ci in range(DCH):
                        nc.tensor.matmul(vt_ps, wv_sb[:, ci, :], xsl(ci),
                                         start=(ci == 0), stop=(ci == DCH - 1))
                    vts = vtsp.tile([128, 512], bf, tag="vts")
                    nc.vector.tensor_copy(vts, vt_ps)

                # PE-transpose V^T to token-major via identity matmuls —
                # deferred one tile so the transposes never wait on vts
                def vt_close(vts=vts, v_t=v_t, tt=tt):
                    for j in range(4):
                        tp = psA.tile([128, 128], bf, tag="a", name="tp")
                        nc.tensor.transpose(
                            tp, vts[:, j * 128:(j + 1) * 128], ident)
                        nc.vector.tensor_copy(v_t[:, tt * 4 + j, :], tp)
                if tt < QT - 1:
                    emit_vt()
                    pending_vt[0] = vt_close
                else:
                    deferred.append(vt_close)

            # ---------------- ATTENTION SWEEP ----------------
            for tt in range(QT):
                t0 = b * S + tt * 512
                tloc = tt * 512
                groups = plan[tt]
                q_cur = q_all[tt]
                mg = [mgp.tile([128, 512], bf, tag=f"mg{h}", name=f"mg{h}")
                      for h in range(HLOC)]
                # pace filler emission evenly across this tile's chunks so
                # the PE doesn't starve at ACT rate once fillers run out
                chunks_total = HLOC * 2 * len(groups)
                ftarget = len(pending)
                fstate = [0, 0]  # chunks done, fillers emitted

                def paced_filler():
                    fstate[0] += 1
                    want = min(ftarget, ftarget * fstate[0] // chunks_total
                               + 1)
                    while fstate[1] < want and pending:
                        pending.popleft()()
                        fstate[1] += 1

                for h in range(HLOC):
                    o_ps = ps_o.tile([128, 512], f32, tag="o")
                    den = dnp.tile([128, 512], f32, tag="den")
                    # flatten groups to chunks, diagonal/masked chunks first
                    chunks = []
                    for (kg, mode, mi) in groups:
                        for i2 in range(2):
                            chunks.append((2 * kg + i2, mode, mi, i2))
                    chunks.sort(key=lambda c: 0 if c[1] != PLAIN else 1)
                    nch = len(chunks)
                    PD = 6  # AV software-pipeline depth
                    pend = []

                    def flush_av(ci_, nch=nch, pend=pend, o_ps=o_ps):
                        kcc, pt, w, qlo = pend[ci_]
                        nc.tensor.matmul(o_ps[:, qlo:512],
                                         v_t[:, kcc, :], pt,
                                         start=(ci_ == 0),
                                         stop=(ci_ == nch - 1),
                                         skip_group_check=True)

                    for ci_, (kcc, mode, mi, i2) in enumerate(chunks):
                        mk = None
                        if mode == MIXED_DATA:
                            mk = mkp.tile([128, 2, 512], f32, tag="mk")
                            nc.sync.dma_start(out=mk[:, i2, :],
                                              in_=mtl[mi, :, i2, :])
                        # causal diagonal chunks: columns q < qlo are
                        # fully masked -> compute only [qlo:512]
                        qlo = 0
                        if mode == MIXED_CAUSAL:
                            qlo = min(max(128 * kcc - tloc, 0), 384)
                        w = 512 - qlo
                        # chunk 0 must cover the full width so PSUM/den
                        # initialization (start/copy) touches every column
                        if ci_ == 0:
                            qlo, w = 0, 512
                        s_ps = psA.tile([128, 512], f32, tag="a", name="s_ps")
                        nc.tensor.matmul(
                            s_ps[:, 0:w],
                            k_t[:, kcc * 128:(kcc + 1) * 128],
                            q_cur[:, h, qlo:512],
                            start=True, stop=True)
                        if mode == MIXED_DATA:
                            nc.vector.tensor_add(s_ps, s_ps, mk[:, i2, :])
                        pt = ptp.tile([128, 512], bf, tag="pt")
                        pt = pt[:, 0:w]
                        nc.scalar.activation(pt, s_ps[:, 0:w],
                                             Exp, scale=SCALE)
                        if mode == MIXED_CAUSAL:
                            # keep iff (qlo+col) - k >= 0 else 0
                            nc.gpsimd.affine_select(
                                out=pt, in_=pt,
                                pattern=[[1, w]],
                                compare_op=mybir.AluOpType.is_ge,
                                fill=0.0,
                                base=tloc + qlo - 128 * kcc,
                                channel_multiplier=-1)
                        # denominator accumulates on DVE, off the PE
                        if ci_ == 0:
                            nc.vector.tensor_copy(den, pt)
                        else:
                            nc.vector.tensor_add(den[:, qlo:512],
                                                 den[:, qlo:512], pt)
                        pend.append((kcc, pt, w, qlo))
                        if ci_ >= PD:
                            flush_av(ci_ - PD)
                        if ci_ == 1:
                            emit_deferred()
                        paced_filler()
                    for ci_ in range(max(0, nch - PD), nch):
                        flush_av(ci_)

                    def tail(h=h, den=den, o_ps=o_ps, mg=mg):
                        # all-ones bf16 matmul: reduces den across
                        # partitions AND broadcasts the sum
                        den_bf = dnp.tile([128, 512], bf, tag="denbf",
                                          bufs=2)
                        nc.vector.tensor_copy(den_bf, den)
                        bc_ps = ps_c.tile([128, 512], f32, tag="o2",
                                          name="bc_ps")
                        nc.tensor.matmul(bc_ps, ones128, den_bf,
                                         start=True, stop=True)
                        rec = recp.tile([128, 512], f32, tag="rec")
                        nc.vector.reciprocal_approx_fast(rec, bc_ps)
                        nc.vector.tensor_mul(mg[h], o_ps, rec)
                    deferred.append(tail)

                # tails must run before this tile's outproj fillers are
                # reachable (they read mg)
                emit_deferred()

                # enqueue output projection for this 512-token tile
                # (partial over this core's 4 heads)
                for og in range(8):
                    holder = {}
                    for tk in range(4):
                        def filler(dma_eng=None, og=og, tk=tk, mg=mg, t0=t0,
                                   holder=holder):
                            o2 = ps_c.tile([128, 512], f32, tag="o2",
                                           name="o2")
                            for h in range(HLOC):
                                nc.tensor.matmul(
                                    o2, mg[h][:, tk * 128:(tk + 1) * 128],
                                    wol_sb[:, h, og * 512:(og + 1) * 512],
                                    start=(h == 0), stop=(h == HLOC - 1))
                            if tk == 0:
                                holder['ot'] = outp.tile([128, 4, 512], bf,
                                                         tag="outsb",
                                                         name="ot", bufs=3)
                            ot_sb = holder['ot']
                            # alternate the PSUM eviction between DVE and
                            # ACT so neither engine saturates in the sweep
                            if (og * 4 + tk) % 2 == 0:
                                nc.vector.tensor_copy(ot_sb[:, tk, :], o2)
                            else:
                                nc.scalar.copy(ot_sb[:, tk, :], o2)
                            if tk == 3:
                                # one merged DMA per og column; issued from
                                # the GpSimd (SWDGE) queue so the HWDGE
                                # queues stay free for x/weight prefetch —
                                # except the final drain, which uses the
                                # by-then-idle HWDGE queues
                                tb0 = t0 // 128
                                eng = dma_eng if dma_eng is not None \
                                    else nc.gpsimd
                                eng.dma_start(
                                    out=out_r[:, tb0:tb0 + 4,
                                              og * 512:(og + 1) * 512],
                                    in_=ot_sb)
                        pending.append(filler)

        # drain remaining output-projection groups (last tile); route their
        # DMAs to the now-idle HWDGE queues
        di = 0
        while pending:
            pending.popleft()(nc.sync if di % 2 == 0 else nc.scalar)
            di += 1


_CACHE = {}


def _get_nc(plan_key, plan, n_mtiles):
    if plan_key not in _CACHE:
        _CACHE[plan_key] = _build(plan, n_mtiles)
    return _CACHE[plan_key]


def _prep_inputs(x, freqs_cis, mask, wq, wk, wv, wo):
    x = np.asarray(x, np.float32).reshape(T, D)
    wq = np.asarray(wq, np.float32)
    wk = np.asarray(wk, np.float32)
    wv = np.asarray(wv, np.float32)
    wo = np.asarray(wo, np.float32)
    fc = np.asarray(freqs_cis, np.float32)

    xt = np.ascontiguousarray(x.T).astype(BF16)            # (D, T)
    permq = _pair_split_perm(N_HEADS)
    permk = _pair_split_perm(N_KV)
    wq_p = wq[permq]
    wk_p = wk[permk]

    cos = fc[:, :, 0].T                                    # (64, S)
    sin = fc[:, :, 1].T
    csa = np.concatenate([cos, cos], axis=0).astype(BF16)  # (128, S)
    csb = np.concatenate([-sin, sin], axis=0).astype(BF16)

    plan, mtiles = _classify_mask(mask)

    in_maps = []
    for c in range(NCORES):
        wqt_c = np.ascontiguousarray(wq_p[c * E:(c + 1) * E].T).astype(BF16)
        wkt_c = np.ascontiguousarray(wk_p[c * HD:(c + 1) * HD].T).astype(BF16)
        wvt_c = np.ascontiguousarray(wv[c * HD:(c + 1) * HD].T).astype(BF16)
        wol_c = np.ascontiguousarray(wo[:, c * E:(c + 1) * E].T).astype(BF16)
        in_maps.append({
            "xt": xt, "wqt": wqt_c, "wkt": wkt_c, "wvt": wvt_c,
            "wol": wol_c, "csa": csa, "csb": csb, "mtl": mtiles,
        })
    return in_maps, plan, mtiles


def kernel(x, freqs_cis, mask, wq, wk, wv, wo, start_pos=0, **_unused):
    in_maps, plan, mtiles = _prep_inputs(x, freqs_cis, mask, wq, wk, wv, wo)
    plan_key = (tuple(tuple(r) for r in plan), mtiles.shape[0])
    nc = _get_nc(plan_key, plan, mtiles.shape[0])
    last_err = None
    for _attempt in range(3):
        try:
            res = run_bass_kernel_spmd(nc, in_maps,
                                       core_ids=list(range(NCORES)))
            break
        except Exception as e:  # transient NRT device errors — retry
            last_err = e
    else:
        raise last_err
    full = np.zeros((T, D), np.float32)
    for c in range(NCORES):
        full += np.asarray(res.results[c]["out"], np.float32)
    return full.reshape(B, S, D).astype(np.float32)


# revision 21
# speedup vs baseline: 1.1640x; 1.1640x over previous
"""Trainium2 Bass kernel for nn_MultiHeadAttention (dense transformer prefill,
GQA 32q/8kv heads, RoPE, causal mask), tensor-parallel over heads across 8
NeuronCores with NO device collective: each core emits a full-shape partial
output (its 4 q heads contracted against the matching 512 columns of wo) and
the host sums the 8 partials during the gather step.

v4 over v3 (820us -> target ~700us):
  - Per-batch two-phase schedule: all 4 tiles' projections emitted as one
    dense GEMM block, then an attention sweep where the previous tile's
    output-projection MM groups are interleaved as PE filler work between
    attention chunks (pending queue carried across tiles AND batches), so
    the PE never idles while ACT grinds the exp chain.
  - All PSUM evictions moved off ACT (was 48% busy inside the exp critical
    chain) onto DVE; softmax denominator accumulation moved to the idle
    GpSimd engine; ACT now does exp only.
  - mg = o * rec multiplies straight out of PSUM on DVE (no o_sb ACT copy);
    per-head softmax tail (den_bf -> ones-matmul -> recip -> mul) deferred
    into the next head's chunk stream so the PE ones-matmul never stalls.
  - Single merged PSUM pool (4 banks) for proj streams + scores + V
    transposes; 2 banks for attention out; 2 for outproj/ones broadcast.
  - Tile-0 startup: K/V/Q0/Q1 streams quarter-interleaved with the x and
    weight quarter DMAs so the PE ramps at DMA delivery rate.
"""

import math
from collections import deque

import numpy as np
import ml_dtypes

import concourse.bass as bass
import concourse.tile as tile
from concourse import bacc, mybir
from concourse import masks
from concourse.bass_utils import run_bass_kernel_spmd

BF16 = ml_dtypes.bfloat16

# ---- problem constants ----
B = 2
S = 2048
D = 4096
N_HEADS = 32
N_KV = 8
HD = 128
NCORES = 8
T = B * S                      # 4096 flattened tokens
HLOC = N_HEADS // NCORES       # 4 local q heads
E = HLOC * HD                  # 512 local q dim
DCH = D // 128                 # 32 contraction chunks
KC = S // 128                  # 16 k-chunks per sequence
KG = KC // 2                   # 8 k-groups (2 chunks) per sequence
QT = S // 512                  # 4 q-tiles per sequence
SCALE = 1.0 / math.sqrt(HD)

SKIP, PLAIN, MIXED_CAUSAL, MIXED_DATA = 0, 1, 2, 3


def _classify_mask(mask):
    """Classify (k-group, q-tile) tiles of mask^T. Returns (plan, mtiles).

    plan[qt] = list of (kg, mode, mtile_idx); mtiles: (n, 128, 2, 512) f32."""
    m = np.asarray(mask, np.float32).reshape(S, S)  # [q, k]
    mt = np.ascontiguousarray(m.T)  # [k, q]
    kk = np.arange(S)
    qq = np.arange(S)
    plan = []
    uniq = {}
    mtiles = []
    for qt in range(QT):
        row = []
        qs = slice(qt * 512, (qt + 1) * 512)
        for kg in range(KG):
            ks = slice(kg * 256, (kg + 1) * 256)
            sub = mt[ks, qs]
            if np.all(sub <= -1e8):
                continue  # fully masked -> skip the whole k-group
            if np.all(sub == 0.0):
                row.append((kg, PLAIN, -1))
                continue
            # mixed: exact-causal check (0 where q>=k, <=-1e8 where q<k)
            causal = qq[None, qs] >= kk[ks, None]
            if np.all((sub == 0.0) == causal) and np.all(sub[~causal] <= -1e8):
                row.append((kg, MIXED_CAUSAL, -1))
            else:
                key = sub.tobytes()
                if key not in uniq:
                    uniq[key] = len(mtiles)
                    mtiles.append(sub.reshape(2, 128, 512).transpose(1, 0, 2))
                row.append((kg, MIXED_DATA, uniq[key]))
        plan.append(row)
    if mtiles:
        mtiles = np.ascontiguousarray(np.stack(mtiles), np.float32)
    else:
        mtiles = np.zeros((1, 128, 2, 512), np.float32)
    return plan, mtiles


def _pair_split_perm(nheads):
    """Row permutation putting each head's even components first, odds second."""
    idx = []
    for h in range(nheads):
        base = h * HD
        idx.extend(base + np.arange(0, HD, 2))
        idx.extend(base + np.arange(1, HD, 2))
    return np.asarray(idx)


def _build(plan, n_mtiles):
    nc = bacc.Bacc("TRN2", target_bir_lowering=False, debug=False,
                   num_devices=NCORES)
    f32, bf = mybir.dt.float32, mybir.dt.bfloat16

    xt = nc.dram_tensor("xt", [D, T], bf, kind="ExternalInput").ap()
    wqt = nc.dram_tensor("wqt", [D, E], bf, kind="ExternalInput").ap()
    wkt = nc.dram_tensor("wkt", [D, HD], bf, kind="ExternalInput").ap()
    wvt = nc.dram_tensor("wvt", [D, HD], bf, kind="ExternalInput").ap()
    wol = nc.dram_tensor("wol", [E, D], bf, kind="ExternalInput").ap()
    csa = nc.dram_tensor("csa", [128, S], bf, kind="ExternalInput").ap()
    csb = nc.dram_tensor("csb", [128, S], bf, kind="ExternalInput").ap()
    mtl = nc.dram_tensor("mtl", [n_mtiles, 128, 2, 512], f32,
                         kind="ExternalInput").ap()
    out = nc.dram_tensor("out", [T, D], bf, kind="ExternalOutput").ap()

    with tile.TileContext(nc) as tc:
        _emit(nc, tc, plan, xt, wqt, wkt, wvt, wol, csa, csb, mtl, out)
    nc.compile()
    return nc


def _emit(nc, tc, plan, xt, wqt, wkt, wvt, wol, csa, csb, mtl, out):
    f32, bf = mybir.dt.float32, mybir.dt.bfloat16
    Exp = mybir.ActivationFunctionType.Exp

    xt_r = xt.rearrange("(c p) t -> p c t", p=128)
    out_r = out.rearrange("(tb p) d -> p tb d", p=128)
    wqt_r = wqt.rearrange("(c p) e -> p c e", p=128)
    wkt_r = wkt.rearrange("(c p) e -> p c e", p=128)
    wvt_r = wvt.rearrange("(c p) e -> p c e", p=128)
    wol_r = wol.rearrange("(h p) d -> p h d", p=128)

    QC = DCH // 4   # 8 contraction chunks per x quarter

    from contextlib import ExitStack
    with ExitStack() as stk:
        singles = stk.enter_context(tc.tile_pool(name="singles", bufs=1))
        xts = stk.enter_context(tc.tile_pool(name="xts", bufs=6))
        kvp = stk.enter_context(tc.tile_pool(name="kv", bufs=1))
        qcp = stk.enter_context(tc.tile_pool(name="qc", bufs=1))
        vtsp = stk.enter_context(tc.tile_pool(name="vts", bufs=2))
        ptp = stk.enter_context(tc.tile_pool(name="ptp", bufs=8))
        rp = stk.enter_context(tc.tile_pool(name="rp", bufs=2))
        dnp = stk.enter_context(tc.tile_pool(name="dna", bufs=3))
        recp = stk.enter_context(tc.tile_pool(name="recp", bufs=2))
        mgp = stk.enter_context(tc.tile_pool(name="mgp", bufs=2))
        mkp = stk.enter_context(tc.tile_pool(name="mkp", bufs=2))
        outp = stk.enter_context(tc.tile_pool(name="outp", bufs=4))
        # PSUM: psA (proj streams + scores + V-transposes) 4 banks,
        # ps_o (attention out) 2, ps_c (outproj groups + ones bcast) 2.
        psA = stk.enter_context(tc.tile_pool(name="psA", bufs=3, space="PSUM"))
        ps_o = stk.enter_context(
            tc.tile_pool(name="ps_o", bufs=3, space="PSUM"))
        ps_c = stk.enter_context(
            tc.tile_pool(name="ps_c", bufs=2, space="PSUM"))

        # --- resident weights / tables (startup-ordered DMAs) ---
        wq_sb = singles.tile([128, DCH, E], bf)
        wk_sb = singles.tile([128, DCH, HD], bf)
        wv_sb = singles.tile([128, DCH, HD], bf)
        wol_sb = singles.tile([128, HLOC, D], bf)
        csa_sb = singles.tile([128, S], bf)
        csb_sb = singles.tile([128, S], bf)
        ones128 = singles.tile([128, 128], bf)
        nc.vector.memset(ones128, 1.0)
        ident = singles.tile([128, 128], bf)
        masks.make_identity(nc, ident)

        def load_weight_quarter(i4):
            nc.sync.dma_start(out=wk_sb[:, i4 * QC:(i4 + 1) * QC, :],
                              in_=wkt_r[:, i4 * QC:(i4 + 1) * QC, :])
            nc.sync.dma_start(out=wv_sb[:, i4 * QC:(i4 + 1) * QC, :],
                              in_=wvt_r[:, i4 * QC:(i4 + 1) * QC, :])
            nc.sync.dma_start(out=wq_sb[:, i4 * QC:(i4 + 1) * QC, :],
                              in_=wqt_r[:, i4 * QC:(i4 + 1) * QC, :])

        def load_rope_tables():
            nc.sync.dma_start(out=csa_sb, in_=csa)
            nc.sync.dma_start(out=csb_sb, in_=csb)

        def load_late_weights():
            nc.sync.dma_start(out=wol_sb, in_=wol_r)

        def apply_rope(ps_tile, dst, tloc, width):
            """dst = RoPE(ps_tile) using csa/csb tables at in-seq pos tloc."""
            ta = rp.tile([128, 512], bf, tag="ta")
            ta = ta[:, :width]
            nc.vector.tensor_mul(ta, ps_tile, csa_sb[:, tloc:tloc + width])
            tb = rp.tile([128, 512], bf, tag="tb")
            tb = tb[:, :width]
            nc.vector.tensor_mul(tb[0:64, :], ps_tile[64:128, :],
                                 csb_sb[0:64, tloc:tloc + width])
            nc.vector.tensor_mul(tb[64:128, :], ps_tile[0:64, :],
                                 csb_sb[64:128, tloc:tloc + width])
            nc.vector.tensor_add(dst, ta, tb)

        # deferred per-head softmax tails; emitted a couple of chunks into
        # the next head so the PE ones-matmul never stalls on the den chain
        deferred = []

        def emit_deferred():
            while deferred:
                deferred.pop()()

        # pending output-projection MM groups (PE filler work); carried
        # across tiles and batches
        pending = deque()

        def emit_filler():
            if pending:
                pending.popleft()()

        # V-transpose work deferred by one tile so the PE transposes never
        # wait on the vts DVE eviction
        pending_vt = [None]

        def emit_vt():
            if pending_vt[0] is not None:
                pending_vt[0]()
                pending_vt[0] = None

        for b in range(B):
            # ---------------- PROJ PHASE (4 q-tiles) ----------------
            k_t = kvp.tile([128, S], bf, tag="k")
            v_t = kvp.tile([128, KC, HD], bf, tag="v")
            q_all = []

            for tt in range(QT):
                t0 = b * S + tt * 512
                tloc = tt * 512
                xh = [xts.tile([128, QC, 512], bf, tag="x", name=f"xh{i}")
                      for i in range(4)]
                if b == 0 and tt == 0:
                    # startup: x slices on the Sync HWDGE queue, wq slices
                    # on the ACT HWDGE queue — two queues spin up and
                    # transfer in parallel; wk/wv (slow 256B-line DMAs)
                    # off the critical path (K/V streams run last)
                    for s in range(QC // 2):
                        sl = slice(2 * s, 2 * s + 2)
                        nc.sync.dma_start(out=xh[0][:, sl, :],
                                          in_=xt_r[:, sl, t0:t0 + 512])
                        nc.sync.dma_start(out=wq_sb[:, sl, :],
                                           in_=wqt_r[:, sl, :])
                    load_rope_tables()
                    for i4 in range(1, 4):
                        for hs in range(2):
                            sl = slice(i4 * QC + hs * (QC // 2),
                                       i4 * QC + (hs + 1) * (QC // 2))
                            nc.sync.dma_start(
                                out=xh[i4][:, hs * (QC // 2):
                                           (hs + 1) * (QC // 2), :],
                                in_=xt_r[:, sl, t0:t0 + 512])
                        nc.sync.dma_start(
                            out=wq_sb[:, i4 * QC:(i4 + 1) * QC, :],
                            in_=wqt_r[:, i4 * QC:(i4 + 1) * QC, :])
                    nc.sync.dma_start(out=wk_sb, in_=wkt_r)
                    nc.sync.dma_start(out=wv_sb, in_=wvt_r)
                    load_late_weights()
                else:
                    for i4 in range(4):
                        for hs in range(2):
                            sl = slice(i4 * QC + hs * (QC // 2),
                                       i4 * QC + (hs + 1) * (QC // 2))
                            nc.sync.dma_start(
                                out=xh[i4][:, hs * (QC // 2):
                                           (hs + 1) * (QC // 2), :],
                                in_=xt_r[:, sl, t0:t0 + 512])

                def xsl(ci):
                    return xh[ci // QC][:, ci % QC, :]

                q_cur = qcp.tile([128, HLOC, 512], bf, tag=f"qc{tt}",
                                 name=f"qc{tt}")
                q_all.append(q_cur)

                if b == 0 and tt == 0:
                    # quarter-major interleave of the 4 Q streams so the PE
                    # ramps at DMA delivery rate (wq has fast 1KB lines);
                    # K/V streams follow once wk/wv have landed
                    q_pss = [psA.tile([128, 512], f32, tag="a",
                                      name=f"q{h}_ps") for h in range(3)]
                    q_pss.append(ps_o.tile([128, 512], f32, tag="o",
                                           name="q3_ps"))
                    for i4 in range(4):
                        for ci in range(i4 * QC, (i4 + 1) * QC):
                            st, sp = (ci == 0), (ci == DCH - 1)
                            for h in range(HLOC):
                                nc.tensor.matmul(
                                    q_pss[h], wq_sb[:, ci, h * HD:(h + 1) * HD],
                                    xsl(ci), start=st, stop=sp,
                                    skip_group_check=True)
                    for h in range(HLOC):
                        apply_rope(q_pss[h], q_cur[:, h, :], tloc, 512)
                    # K projection + RoPE
                    k_ps = psA.tile([128, 512], f32, tag="a", name="k_ps")
                    for ci in range(DCH):
                        nc.tensor.matmul(k_ps, wk_sb[:, ci, :], xsl(ci),
                                         start=(ci == 0), stop=(ci == DCH - 1))
                    apply_rope(k_ps, k_t[:, tloc:tloc + 512], tloc, 512)
                    # V^T projection
                    vt_ps = psA.tile([128, 512], f32, tag="a", name="vt_ps")
                    for ci in range(DCH):
                        nc.tensor.matmul(vt_ps, wv_sb[:, ci, :], xsl(ci),
                                         start=(ci == 0), stop=(ci == DCH - 1))
                    vts = vtsp.tile([128, 512], bf, tag="vts")
                    nc.vector.tensor_copy(vts, vt_ps)
                else:
                    # Q projections + RoPE
                    for h in range(HLOC):
                        q_ps = psA.tile([128, 512], f32, tag="a", name="q_ps")
                        for ci in range(DCH):
                            nc.tensor.matmul(
                                q_ps, wq_sb[:, ci, h * HD:(h + 1) * HD],
                                xsl(ci), start=(ci == 0), stop=(ci == DCH - 1))
                        apply_rope(q_ps, q_cur[:, h, :], tloc, 512)
                        if h == 0:
                            emit_vt()
                    # K projection + RoPE
                    k_ps = psA.tile([128, 512], f32, tag="a", name="k_ps")
                    for ci in range(DCH):
                        nc.tensor.matmul(k_ps, wk_sb[:, ci, :], xsl(ci),
                                         start=(ci == 0), stop=(ci == DCH - 1))
                    apply_rope(k_ps, k_t[:, tloc:tloc + 512], tloc, 512)
                    # V^T projection (e-major wide matmuls)
                    vt_ps = psA.tile([128, 512], f32, tag="a", name="vt_ps")
                    for ci in range(DCH):
                        nc.tensor.matmul(vt_ps, wv_sb[:, ci, :], xsl(ci),
                                         start=(ci == 0), stop=(ci == DCH - 1))
                    vts = vtsp.tile([128, 512], bf, tag="vts")
                    nc.vector.tensor_copy(vts, vt_ps)

                # PE-transpose V^T to token-major via identity matmuls —
                # deferred one tile so the transposes never wait on vts
                def vt_close(vts=vts, v_t=v_t, tt=tt):
                    for j in range(4):
                        tp = psA.tile([128, 128], bf, tag="a", name="tp")
                        nc.tensor.transpose(
                            tp, vts[:, j * 128:(j + 1) * 128], ident)
                        nc.vector.tensor_copy(v_t[:, tt * 4 + j, :], tp)
                if tt < QT - 1:
                    emit_vt()
                    pending_vt[0] = vt_close
                else:
                    deferred.append(vt_close)

            # ---------------- ATTENTION SWEEP ----------------
            for tt in range(QT):
                t0 = b * S + tt * 512
                tloc = tt * 512
                groups = plan[tt]
                q_cur = q_all[tt]
                mg = [mgp.tile([128, 512], bf, tag=f"mg{h}", name=f"mg{h}")
                      for h in range(HLOC)]
                # pace filler emission evenly across this tile's chunks so
                # the PE doesn't starve at ACT rate once fillers run out
                chunks_total = HLOC * 2 * len(groups)
                ftarget = len(pending)
                fstate = [0, 0]  # chunks done, fillers emitted

                def paced_filler():
                    fstate[0] += 1
                    want = min(ftarget,
                               ftarget * max(0, fstate[0] - 2)
                               // max(1, chunks_total - 2))
                    while fstate[1] < want and pending:
                        pending.popleft()()
                        fstate[1] += 1

                for h in range(HLOC):
                    o_ps = ps_o.tile([128, 512], f32, tag="o")
                    den = dnp.tile([128, 512], f32, tag="den")
                    # flatten groups to chunks, diagonal/masked chunks first
                    chunks = []
                    for (kg, mode, mi) in groups:
                        for i2 in range(2):
                            chunks.append((2 * kg + i2, mode, mi, i2))
                    chunks.sort(key=lambda c: 0 if c[1] != PLAIN else 1)
                    nch = len(chunks)
                    PD = 6  # AV software-pipeline depth
                    pend = []

                    def flush_av(ci_, nch=nch, pend=pend, o_ps=o_ps):
                        kcc, pt, w, qlo = pend[ci_]
                        nc.tensor.matmul(o_ps[:, qlo:512],
                                         v_t[:, kcc, :], pt,
                                         start=(ci_ == 0),
                                         stop=(ci_ == nch - 1),
                                         skip_group_check=True)

                    for ci_, (kcc, mode, mi, i2) in enumerate(chunks):
                        mk = None
                        if mode == MIXED_DATA:
                            mk = mkp.tile([128, 2, 512], f32, tag="mk")
                            nc.sync.dma_start(out=mk[:, i2, :],
                                              in_=mtl[mi, :, i2, :])
                        # causal diagonal chunks: columns q < qlo are
                        # fully masked -> compute only [qlo:512]
                        qlo = 0
                        if mode == MIXED_CAUSAL:
                            qlo = min(max(128 * kcc - tloc, 0), 384)
                        w = 512 - qlo
                        # chunk 0 must cover the full width so PSUM/den
                        # initialization (start/copy) touches every column
                        if ci_ == 0:
                            qlo, w = 0, 512
                        s_ps = psA.tile([128, 512], f32, tag="a", name="s_ps")
                        nc.tensor.matmul(
                            s_ps[:, 0:w],
                            k_t[:, kcc * 128:(kcc + 1) * 128],
                            q_cur[:, h, qlo:512],
                            start=True, stop=True)
                        if mode == MIXED_DATA:
                            nc.vector.tensor_add(s_ps, s_ps, mk[:, i2, :])
                        pt = ptp.tile([128, 512], bf, tag="pt")
                        pt = pt[:, 0:w]
                        nc.scalar.activation(pt, s_ps[:, 0:w],
                                             Exp, scale=SCALE)
                        if mode == MIXED_CAUSAL:
                            # keep iff (qlo+col) - k >= 0 else 0
                            nc.gpsimd.affine_select(
                                out=pt, in_=pt,
                                pattern=[[1, w]],
                                compare_op=mybir.AluOpType.is_ge,
                                fill=0.0,
                                base=tloc + qlo - 128 * kcc,
                                channel_multiplier=-1)
                        # denominator accumulates on DVE, off the PE
                        if ci_ == 0:
                            nc.vector.tensor_copy(den, pt)
                        else:
                            nc.vector.tensor_add(den[:, qlo:512],
                                                 den[:, qlo:512], pt)
                        pend.append((kcc, pt, w, qlo))
                        if ci_ >= PD:
                            flush_av(ci_ - PD)
                        if ci_ == 1:
                            emit_deferred()
                        paced_filler()
                    for ci_ in range(max(0, nch - PD), nch):
                        flush_av(ci_)

                    def tail(h=h, den=den, o_ps=o_ps, mg=mg):
                        # all-ones bf16 matmul: reduces den across
                        # partitions AND broadcasts the sum
                        den_bf = dnp.tile([128, 512], bf, tag="denbf",
                                          bufs=2)
                        nc.vector.tensor_copy(den_bf, den)
                        bc_ps = ps_c.tile([128, 512], f32, tag="o2",
                                          name="bc_ps")
                        nc.tensor.matmul(bc_ps, ones128, den_bf,
                                         start=True, stop=True)
                        rec = recp.tile([128, 512], f32, tag="rec")
                        nc.vector.reciprocal_approx_fast(rec, bc_ps)
                        nc.vector.tensor_mul(mg[h], o_ps, rec)
                    deferred.append(tail)

                # enqueue output projection for this 512-token tile
                # (partial over this core's 4 heads)
                for og in range(8):
                    holder = {}
                    for tk in range(4):
                        def filler(dma_eng=None, og=og, tk=tk, mg=mg, t0=t0,
                                   holder=holder):
                            o2 = ps_c.tile([128, 512], f32, tag="o2",
                                           name="o2")
                            for h in range(HLOC):
                                nc.tensor.matmul(
                                    o2, mg[h][:, tk * 128:(tk + 1) * 128],
                                    wol_sb[:, h, og * 512:(og + 1) * 512],
                                    start=(h == 0), stop=(h == HLOC - 1))
                            if tk == 0:
                                holder['ot'] = outp.tile([128, 4, 512], bf,
                                                         tag="outsb",
                                                         name="ot", bufs=3)
                            ot_sb = holder['ot']
                            # alternate the PSUM eviction between DVE and
                            # ACT so neither engine saturates in the sweep
                            if (og * 4 + tk) % 2 == 0:
                                nc.vector.tensor_copy(ot_sb[:, tk, :], o2)
                            else:
                                nc.scalar.copy(ot_sb[:, tk, :], o2)
                            if tk == 3:
                                # one merged DMA per og column; issued from
                                # the GpSimd (SWDGE) queue so the HWDGE
                                # queues stay free for x/weight prefetch —
                                # except the final drain, which uses the
                                # by-then-idle HWDGE queues
                                tb0 = t0 // 128
                                eng = dma_eng if dma_eng is not None \
                                    else nc.gpsimd
                                eng.dma_start(
                                    out=out_r[:, tb0:tb0 + 4,
                                              og * 512:(og + 1) * 512],
                                    in_=ot_sb)
                        pending.append(filler)

        # drain remaining tails, then the last tile's output-projection
        # groups; route half their DMAs to the now-idle Sync HWDGE queue
        emit_deferred()
        di = 0
        while pending:
            pending.popleft()(nc.sync if di % 2 == 0 else None)
            di += 1


_CACHE = {}


def _get_nc(plan_key, plan, n_mtiles):
    if plan_key not in _CACHE:
        _CACHE[plan_key] = _build(plan, n_mtiles)
    return _CACHE[plan_key]


def _prep_inputs(x, freqs_cis, mask, wq, wk, wv, wo):
    x = np.asarray(x, np.float32).reshape(T, D)
    wq = np.asarray(wq, np.float32)
    wk = np.asarray(wk, np.float32)
    wv = np.asarray(wv, np.float32)
    wo = np.asarray(wo, np.float32)
    fc = np.asarray(freqs_cis, np.float32)

    xt = np.ascontiguousarray(x.T).astype(BF16)            # (D, T)
    permq = _pair_split_perm(N_HEADS)
    permk = _pair_split_perm(N_KV)
    wq_p = wq[permq]
    wk_p = wk[permk]

    cos = fc[:, :, 0].T                                    # (64, S)
    sin = fc[:, :, 1].T
    csa = np.concatenate([cos, cos], axis=0).astype(BF16)  # (128, S)
    csb = np.concatenate([-sin, sin], axis=0).astype(BF16)

    plan, mtiles = _classify_mask(mask)

    in_maps = []
    for c in range(NCORES):
        wqt_c = np.ascontiguousarray(wq_p[c * E:(c + 1) * E].T).astype(BF16)
        wkt_c = np.ascontiguousarray(wk_p[c * HD:(c + 1) * HD].T).astype(BF16)
        wvt_c = np.ascontiguousarray(wv[c * HD:(c + 1) * HD].T).astype(BF16)
        wol_c = np.ascontiguousarray(wo[:, c * E:(c + 1) * E].T).astype(BF16)
        in_maps.append({
            "xt": xt, "wqt": wqt_c, "wkt": wkt_c, "wvt": wvt_c,
            "wol": wol_c, "csa": csa, "csb": csb, "mtl": mtiles,
        })
    return in_maps, plan, mtiles


def kernel(x, freqs_cis, mask, wq, wk, wv, wo, start_pos=0, **_unused):
    in_maps, plan, mtiles = _prep_inputs(x, freqs_cis, mask, wq, wk, wv, wo)
    plan_key = (tuple(tuple(r) for r in plan), mtiles.shape[0])
    nc = _get_nc(plan_key, plan, mtiles.shape[0])
    last_err = None
    for _attempt in range(3):
        try:
            res = run_bass_kernel_spmd(nc, in_maps,
                                       core_ids=list(range(NCORES)))
            break
        except Exception as e:  # transient NRT device errors — retry
            last_err = e
    else:
        raise last_err
    full = np.zeros((T, D), np.float32)
    for c in range(NCORES):
        full += np.asarray(res.results[c]["out"], np.float32)
    return full.reshape(B, S, D).astype(np.float32)


# revision 22
# speedup vs baseline: 1.1946x; 1.0263x over previous
"""Trainium2 Bass kernel for nn_MultiHeadAttention (dense transformer prefill,
GQA 32q/8kv heads, RoPE, causal mask), tensor-parallel over heads across 8
NeuronCores with NO device collective: each core emits a full-shape partial
output (its 4 q heads contracted against the matching 512 columns of wo) and
the host sums the 8 partials during the gather step.

v4 over v3 (820us -> target ~700us):
  - Per-batch two-phase schedule: all 4 tiles' projections emitted as one
    dense GEMM block, then an attention sweep where the previous tile's
    output-projection MM groups are interleaved as PE filler work between
    attention chunks (pending queue carried across tiles AND batches), so
    the PE never idles while ACT grinds the exp chain.
  - All PSUM evictions moved off ACT (was 48% busy inside the exp critical
    chain) onto DVE; softmax denominator accumulation moved to the idle
    GpSimd engine; ACT now does exp only.
  - mg = o * rec multiplies straight out of PSUM on DVE (no o_sb ACT copy);
    per-head softmax tail (den_bf -> ones-matmul -> recip -> mul) deferred
    into the next head's chunk stream so the PE ones-matmul never stalls.
  - Single merged PSUM pool (4 banks) for proj streams + scores + V
    transposes; 2 banks for attention out; 2 for outproj/ones broadcast.
  - Tile-0 startup: K/V/Q0/Q1 streams quarter-interleaved with the x and
    weight quarter DMAs so the PE ramps at DMA delivery rate.
"""

import math
from collections import deque

import numpy as np
import ml_dtypes

import concourse.bass as bass
import concourse.tile as tile
from concourse import bacc, mybir
from concourse import masks
from concourse.bass_utils import run_bass_kernel_spmd

BF16 = ml_dtypes.bfloat16

# ---- problem constants ----
B = 2
S = 2048
D = 4096
N_HEADS = 32
N_KV = 8
HD = 128
NCORES = 8
T = B * S                      # 4096 flattened tokens
HLOC = N_HEADS // NCORES       # 4 local q heads
E = HLOC * HD                  # 512 local q dim
DCH = D // 128                 # 32 contraction chunks
KC = S // 128                  # 16 k-chunks per sequence
KG = KC // 2                   # 8 k-groups (2 chunks) per sequence
QT = S // 512                  # 4 q-tiles per sequence
SCALE = 1.0 / math.sqrt(HD)

SKIP, PLAIN, MIXED_CAUSAL, MIXED_DATA = 0, 1, 2, 3


def _classify_mask(mask):
    """Classify (k-group, q-tile) tiles of mask^T. Returns (plan, mtiles).

    plan[qt] = list of (kg, mode, mtile_idx); mtiles: (n, 128, 2, 512) f32."""
    m = np.asarray(mask, np.float32).reshape(S, S)  # [q, k]
    mt = np.ascontiguousarray(m.T)  # [k, q]
    kk = np.arange(S)
    qq = np.arange(S)
    plan = []
    uniq = {}
    mtiles = []
    for qt in range(QT):
        row = []
        qs = slice(qt * 512, (qt + 1) * 512)
        for kg in range(KG):
            ks = slice(kg * 256, (kg + 1) * 256)
            sub = mt[ks, qs]
            if np.all(sub <= -1e8):
                continue  # fully masked -> skip the whole k-group
            if np.all(sub == 0.0):
                row.append((kg, PLAIN, -1))
                continue
            # mixed: exact-causal check (0 where q>=k, <=-1e8 where q<k)
            causal = qq[None, qs] >= kk[ks, None]
            if np.all((sub == 0.0) == causal) and np.all(sub[~causal] <= -1e8):
                row.append((kg, MIXED_CAUSAL, -1))
            else:
                key = sub.tobytes()
                if key not in uniq:
                    uniq[key] = len(mtiles)
                    mtiles.append(sub.reshape(2, 128, 512).transpose(1, 0, 2))
                row.append((kg, MIXED_DATA, uniq[key]))
        plan.append(row)
    if mtiles:
        mtiles = np.ascontiguousarray(np.stack(mtiles), np.float32)
    else:
        mtiles = np.zeros((1, 128, 2, 512), np.float32)
    return plan, mtiles


def _pair_split_perm(nheads):
    """Row permutation putting each head's even components first, odds second."""
    idx = []
    for h in range(nheads):
        base = h * HD
        idx.extend(base + np.arange(0, HD, 2))
        idx.extend(base + np.arange(1, HD, 2))
    return np.asarray(idx)


def _build(plan, n_mtiles):
    nc = bacc.Bacc("TRN2", target_bir_lowering=False, debug=False,
                   num_devices=NCORES)
    f32, bf = mybir.dt.float32, mybir.dt.bfloat16

    xt = nc.dram_tensor("xt", [D, T], bf, kind="ExternalInput").ap()
    wqt = nc.dram_tensor("wqt", [D, E], bf, kind="ExternalInput").ap()
    wkt = nc.dram_tensor("wkt", [D, HD], bf, kind="ExternalInput").ap()
    wvt = nc.dram_tensor("wvt", [D, HD], bf, kind="ExternalInput").ap()
    wol = nc.dram_tensor("wol", [E, D], bf, kind="ExternalInput").ap()
    csa = nc.dram_tensor("csa", [128, S], bf, kind="ExternalInput").ap()
    csb = nc.dram_tensor("csb", [128, S], bf, kind="ExternalInput").ap()
    mtl = nc.dram_tensor("mtl", [n_mtiles, 128, 2, 512], f32,
                         kind="ExternalInput").ap()
    out = nc.dram_tensor("out", [T, D], bf, kind="ExternalOutput").ap()

    with tile.TileContext(nc) as tc:
        _emit(nc, tc, plan, xt, wqt, wkt, wvt, wol, csa, csb, mtl, out)
    nc.compile()
    return nc


def _emit(nc, tc, plan, xt, wqt, wkt, wvt, wol, csa, csb, mtl, out):
    f32, bf = mybir.dt.float32, mybir.dt.bfloat16
    Exp = mybir.ActivationFunctionType.Exp

    xt_r = xt.rearrange("(c p) t -> p c t", p=128)
    out_r = out.rearrange("(tb p) d -> p tb d", p=128)
    wqt_r = wqt.rearrange("(c p) e -> p c e", p=128)
    wkt_r = wkt.rearrange("(c p) e -> p c e", p=128)
    wvt_r = wvt.rearrange("(c p) e -> p c e", p=128)
    wol_r = wol.rearrange("(h p) d -> p h d", p=128)

    QC = DCH // 4   # 8 contraction chunks per x quarter

    from contextlib import ExitStack
    with ExitStack() as stk:
        singles = stk.enter_context(tc.tile_pool(name="singles", bufs=1))
        xts = stk.enter_context(tc.tile_pool(name="xts", bufs=6))
        kvp = stk.enter_context(tc.tile_pool(name="kv", bufs=1))
        qcp = stk.enter_context(tc.tile_pool(name="qc", bufs=1))
        vtsp = stk.enter_context(tc.tile_pool(name="vts", bufs=2))
        ptp = stk.enter_context(tc.tile_pool(name="ptp", bufs=8))
        rp = stk.enter_context(tc.tile_pool(name="rp", bufs=2))
        dnp = stk.enter_context(tc.tile_pool(name="dna", bufs=3))
        recp = stk.enter_context(tc.tile_pool(name="recp", bufs=2))
        mgp = stk.enter_context(tc.tile_pool(name="mgp", bufs=2))
        mkp = stk.enter_context(tc.tile_pool(name="mkp", bufs=2))
        outp = stk.enter_context(tc.tile_pool(name="outp", bufs=4))
        # PSUM: psA (proj streams + scores + V-transposes) 4 banks,
        # ps_o (attention out) 2, ps_c (outproj groups + ones bcast) 2.
        psA = stk.enter_context(tc.tile_pool(name="psA", bufs=4, space="PSUM"))
        ps_o = stk.enter_context(
            tc.tile_pool(name="ps_o", bufs=2, space="PSUM"))
        ps_c = stk.enter_context(
            tc.tile_pool(name="ps_c", bufs=2, space="PSUM"))

        # --- resident weights / tables (startup-ordered DMAs) ---
        wq_sb = singles.tile([128, DCH, E], bf)
        wk_sb = singles.tile([128, DCH, HD], bf)
        wv_sb = singles.tile([128, DCH, HD], bf)
        wol_sb = singles.tile([128, HLOC, D], bf)
        csa_sb = singles.tile([128, S], bf)
        csb_sb = singles.tile([128, S], bf)
        ones128 = singles.tile([128, 128], bf)
        nc.vector.memset(ones128, 1.0)
        ident = singles.tile([128, 128], bf)
        masks.make_identity(nc, ident)

        def load_weight_quarter(i4):
            nc.sync.dma_start(out=wk_sb[:, i4 * QC:(i4 + 1) * QC, :],
                              in_=wkt_r[:, i4 * QC:(i4 + 1) * QC, :])
            nc.sync.dma_start(out=wv_sb[:, i4 * QC:(i4 + 1) * QC, :],
                              in_=wvt_r[:, i4 * QC:(i4 + 1) * QC, :])
            nc.sync.dma_start(out=wq_sb[:, i4 * QC:(i4 + 1) * QC, :],
                              in_=wqt_r[:, i4 * QC:(i4 + 1) * QC, :])

        def load_rope_tables():
            nc.sync.dma_start(out=csa_sb, in_=csa)
            nc.sync.dma_start(out=csb_sb, in_=csb)

        def load_late_weights():
            nc.sync.dma_start(out=wol_sb, in_=wol_r)

        def apply_rope(ps_tile, dst, tloc, width):
            """dst = RoPE(ps_tile) using csa/csb tables at in-seq pos tloc."""
            ta = rp.tile([128, 512], bf, tag="ta")
            ta = ta[:, :width]
            nc.vector.tensor_mul(ta, ps_tile, csa_sb[:, tloc:tloc + width])
            tb = rp.tile([128, 512], bf, tag="tb")
            tb = tb[:, :width]
            nc.vector.tensor_mul(tb[0:64, :], ps_tile[64:128, :],
                                 csb_sb[0:64, tloc:tloc + width])
            nc.vector.tensor_mul(tb[64:128, :], ps_tile[0:64, :],
                                 csb_sb[64:128, tloc:tloc + width])
            nc.vector.tensor_add(dst, ta, tb)

        # deferred per-head softmax tails; emitted a couple of chunks into
        # the next head so the PE ones-matmul never stalls on the den chain
        deferred = []

        def emit_deferred():
            while deferred:
                deferred.pop()()

        # pending output-projection MM groups (PE filler work); carried
        # across tiles and batches
        pending = deque()

        def emit_filler():
            if pending:
                pending.popleft()()

        # V-transpose work deferred by one tile so the PE transposes never
        # wait on the vts DVE eviction
        pending_vt = [None]

        def emit_vt():
            if pending_vt[0] is not None:
                pending_vt[0]()
                pending_vt[0] = None

        for b in range(B):
            # ---------------- PROJ PHASE (4 q-tiles) ----------------
            k_t = kvp.tile([128, S], bf, tag="k")
            v_t = kvp.tile([128, KC, HD], bf, tag="v")
            q_all = []

            for tt in range(QT):
                t0 = b * S + tt * 512
                tloc = tt * 512
                xh = [xts.tile([128, QC, 512], bf, tag="x", name=f"xh{i}")
                      for i in range(4)]
                if b == 0 and tt == 0:
                    # startup: x slices on the Sync HWDGE queue, wq slices
                    # on the ACT HWDGE queue — two queues spin up and
                    # transfer in parallel; wk/wv (slow 256B-line DMAs)
                    # off the critical path (K/V streams run last)
                    for s in range(QC // 2):
                        sl = slice(2 * s, 2 * s + 2)
                        nc.sync.dma_start(out=xh[0][:, sl, :],
                                          in_=xt_r[:, sl, t0:t0 + 512])
                        nc.sync.dma_start(out=wq_sb[:, sl, :],
                                           in_=wqt_r[:, sl, :])
                    load_rope_tables()
                    for i4 in range(1, 4):
                        for hs in range(2):
                            sl = slice(i4 * QC + hs * (QC // 2),
                                       i4 * QC + (hs + 1) * (QC // 2))
                            nc.sync.dma_start(
                                out=xh[i4][:, hs * (QC // 2):
                                           (hs + 1) * (QC // 2), :],
                                in_=xt_r[:, sl, t0:t0 + 512])
                        nc.sync.dma_start(
                            out=wq_sb[:, i4 * QC:(i4 + 1) * QC, :],
                            in_=wqt_r[:, i4 * QC:(i4 + 1) * QC, :])
                    nc.sync.dma_start(out=wk_sb, in_=wkt_r)
                    nc.sync.dma_start(out=wv_sb, in_=wvt_r)
                    load_late_weights()
                else:
                    for i4 in range(4):
                        for hs in range(2):
                            sl = slice(i4 * QC + hs * (QC // 2),
                                       i4 * QC + (hs + 1) * (QC // 2))
                            nc.sync.dma_start(
                                out=xh[i4][:, hs * (QC // 2):
                                           (hs + 1) * (QC // 2), :],
                                in_=xt_r[:, sl, t0:t0 + 512])

                def xsl(ci):
                    return xh[ci // QC][:, ci % QC, :]

                q_cur = qcp.tile([128, HLOC, 512], bf, tag=f"qc{tt}",
                                 name=f"qc{tt}")
                q_all.append(q_cur)

                if b == 0 and tt == 0:
                    # quarter-major interleave of the 4 Q streams so the PE
                    # ramps at DMA delivery rate (wq has fast 1KB lines);
                    # K/V streams follow once wk/wv have landed
                    q_pss = [psA.tile([128, 512], f32, tag="a",
                                      name=f"q{h}_ps") for h in range(HLOC)]
                    for i4 in range(4):
                        for ci in range(i4 * QC, (i4 + 1) * QC):
                            st, sp = (ci == 0), (ci == DCH - 1)
                            for h in range(HLOC):
                                nc.tensor.matmul(
                                    q_pss[h], wq_sb[:, ci, h * HD:(h + 1) * HD],
                                    xsl(ci), start=st, stop=sp,
                                    skip_group_check=True)
                    for h in range(HLOC):
                        apply_rope(q_pss[h], q_cur[:, h, :], tloc, 512)
                    # K projection + RoPE
                    k_ps = psA.tile([128, 512], f32, tag="a", name="k_ps")
                    for ci in range(DCH):
                        nc.tensor.matmul(k_ps, wk_sb[:, ci, :], xsl(ci),
                                         start=(ci == 0), stop=(ci == DCH - 1))
                    apply_rope(k_ps, k_t[:, tloc:tloc + 512], tloc, 512)
                    # V^T projection
                    vt_ps = psA.tile([128, 512], f32, tag="a", name="vt_ps")
                    for ci in range(DCH):
                        nc.tensor.matmul(vt_ps, wv_sb[:, ci, :], xsl(ci),
                                         start=(ci == 0), stop=(ci == DCH - 1))
                    vts = vtsp.tile([128, 512], bf, tag="vts")
                    nc.vector.tensor_copy(vts, vt_ps)
                else:
                    # Q projections + RoPE
                    for h in range(HLOC):
                        q_ps = psA.tile([128, 512], f32, tag="a", name="q_ps")
                        for ci in range(DCH):
                            nc.tensor.matmul(
                                q_ps, wq_sb[:, ci, h * HD:(h + 1) * HD],
                                xsl(ci), start=(ci == 0), stop=(ci == DCH - 1))
                        apply_rope(q_ps, q_cur[:, h, :], tloc, 512)
                        if h == 0:
                            emit_vt()
                    # K projection + RoPE
                    k_ps = psA.tile([128, 512], f32, tag="a", name="k_ps")
                    for ci in range(DCH):
                        nc.tensor.matmul(k_ps, wk_sb[:, ci, :], xsl(ci),
                                         start=(ci == 0), stop=(ci == DCH - 1))
                    apply_rope(k_ps, k_t[:, tloc:tloc + 512], tloc, 512)
                    # V^T projection (e-major wide matmuls)
                    vt_ps = psA.tile([128, 512], f32, tag="a", name="vt_ps")
                    for ci in range(DCH):
                        nc.tensor.matmul(vt_ps, wv_sb[:, ci, :], xsl(ci),
                                         start=(ci == 0), stop=(ci == DCH - 1))
                    vts = vtsp.tile([128, 512], bf, tag="vts")
                    nc.vector.tensor_copy(vts, vt_ps)

                # PE-transpose V^T to token-major via identity matmuls —
                # deferred one tile so the transposes never wait on vts
                def vt_close(vts=vts, v_t=v_t, tt=tt):
                    for j in range(4):
                        tp = psA.tile([128, 128], bf, tag="a", name="tp")
                        nc.tensor.transpose(
                            tp, vts[:, j * 128:(j + 1) * 128], ident)
                        nc.vector.tensor_copy(v_t[:, tt * 4 + j, :], tp)
                if tt < QT - 1:
                    emit_vt()
                    pending_vt[0] = vt_close
                else:
                    deferred.append(vt_close)

            # ---------------- ATTENTION SWEEP ----------------
            for tt in range(QT):
                t0 = b * S + tt * 512
                tloc = tt * 512
                groups = plan[tt]
                q_cur = q_all[tt]
                mg = [mgp.tile([128, 512], bf, tag=f"mg{h}", name=f"mg{h}")
                      for h in range(HLOC)]
                # pace filler emission evenly across this tile's chunks so
                # the PE doesn't starve at ACT rate once fillers run out
                chunks_total = HLOC * 2 * len(groups)
                ftarget = len(pending)
                fstate = [0, 0]  # chunks done, fillers emitted

                def paced_filler():
                    fstate[0] += 1
                    want = min(ftarget, ftarget * fstate[0] // chunks_total
                               + 1)
                    while fstate[1] < want and pending:
                        pending.popleft()()
                        fstate[1] += 1

                for h in range(HLOC):
                    o_ps = ps_o.tile([128, 512], f32, tag="o")
                    den = dnp.tile([128, 512], f32, tag="den")
                    # flatten groups to chunks, diagonal/masked chunks first
                    chunks = []
                    for (kg, mode, mi) in groups:
                        for i2 in range(2):
                            chunks.append((2 * kg + i2, mode, mi, i2))
                    chunks.sort(key=lambda c: 0 if c[1] != PLAIN else 1)
                    nch = len(chunks)
                    PD = 6  # AV software-pipeline depth
                    pend = []

                    def flush_av(ci_, nch=nch, pend=pend, o_ps=o_ps):
                        kcc, pt, w, qlo = pend[ci_]
                        nc.tensor.matmul(o_ps[:, qlo:512],
                                         v_t[:, kcc, :], pt,
                                         start=(ci_ == 0),
                                         stop=(ci_ == nch - 1),
                                         skip_group_check=True)

                    for ci_, (kcc, mode, mi, i2) in enumerate(chunks):
                        mk = None
                        if mode == MIXED_DATA:
                            mk = mkp.tile([128, 2, 512], f32, tag="mk")
                            nc.sync.dma_start(out=mk[:, i2, :],
                                              in_=mtl[mi, :, i2, :])
                        # causal diagonal chunks: columns q < qlo are
                        # fully masked -> compute only [qlo:512]
                        qlo = 0
                        if mode == MIXED_CAUSAL:
                            qlo = min(max(128 * kcc - tloc, 0), 384)
                        w = 512 - qlo
                        # chunk 0 must cover the full width so PSUM/den
                        # initialization (start/copy) touches every column
                        if ci_ == 0:
                            qlo, w = 0, 512
                        s_ps = psA.tile([128, 512], f32, tag="a", name="s_ps")
                        nc.tensor.matmul(
                            s_ps[:, 0:w],
                            k_t[:, kcc * 128:(kcc + 1) * 128],
                            q_cur[:, h, qlo:512],
                            start=True, stop=True)
                        if mode == MIXED_DATA:
                            nc.vector.tensor_add(s_ps, s_ps, mk[:, i2, :])
                        pt = ptp.tile([128, 512], bf, tag="pt")
                        pt = pt[:, 0:w]
                        nc.scalar.activation(pt, s_ps[:, 0:w],
                                             Exp, scale=SCALE)
                        if mode == MIXED_CAUSAL:
                            # keep iff (qlo+col) - k >= 0 else 0
                            nc.gpsimd.affine_select(
                                out=pt, in_=pt,
                                pattern=[[1, w]],
                                compare_op=mybir.AluOpType.is_ge,
                                fill=0.0,
                                base=tloc + qlo - 128 * kcc,
                                channel_multiplier=-1)
                        # denominator accumulates on DVE, off the PE
                        if ci_ == 0:
                            nc.vector.tensor_copy(den, pt)
                        else:
                            nc.vector.tensor_add(den[:, qlo:512],
                                                 den[:, qlo:512], pt)
                        pend.append((kcc, pt, w, qlo))
                        if ci_ >= PD:
                            flush_av(ci_ - PD)
                        if ci_ == 1:
                            emit_deferred()
                        paced_filler()
                    for ci_ in range(max(0, nch - PD), nch):
                        flush_av(ci_)

                    def tail(h=h, den=den, o_ps=o_ps, mg=mg):
                        # all-ones bf16 matmul: reduces den across
                        # partitions AND broadcasts the sum
                        den_bf = dnp.tile([128, 512], bf, tag="denbf",
                                          bufs=2)
                        nc.vector.tensor_copy(den_bf, den)
                        bc_ps = ps_c.tile([128, 512], f32, tag="o2",
                                          name="bc_ps")
                        nc.tensor.matmul(bc_ps, ones128, den_bf,
                                         start=True, stop=True)
                        rec = recp.tile([128, 512], f32, tag="rec")
                        nc.vector.reciprocal_approx_fast(rec, bc_ps)
                        nc.vector.tensor_mul(mg[h], o_ps, rec)
                    deferred.append(tail)

                # tails must run before this tile's outproj fillers are
                # reachable (they read mg)
                emit_deferred()

                # enqueue output projection for this 512-token tile
                # (partial over this core's 4 heads)
                for og in range(8):
                    holder = {}
                    for tk in range(4):
                        def filler(dma_eng=None, og=og, tk=tk, mg=mg, t0=t0,
                                   holder=holder):
                            o2 = ps_c.tile([128, 512], f32, tag="o2",
                                           name="o2")
                            for h in range(HLOC):
                                nc.tensor.matmul(
                                    o2, mg[h][:, tk * 128:(tk + 1) * 128],
                                    wol_sb[:, h, og * 512:(og + 1) * 512],
                                    start=(h == 0), stop=(h == HLOC - 1))
                            if tk == 0:
                                holder['ot'] = outp.tile([128, 4, 512], bf,
                                                         tag="outsb",
                                                         name="ot", bufs=3)
                            ot_sb = holder['ot']
                            # alternate the PSUM eviction between DVE and
                            # ACT so neither engine saturates in the sweep
                            if (og * 4 + tk) % 2 == 0:
                                nc.vector.tensor_copy(ot_sb[:, tk, :], o2)
                            else:
                                nc.scalar.copy(ot_sb[:, tk, :], o2)
                            if tk == 3:
                                # one merged DMA per og column; issued from
                                # the GpSimd (SWDGE) queue so the HWDGE
                                # queues stay free for x/weight prefetch —
                                # except the final drain, which uses the
                                # by-then-idle HWDGE queues
                                tb0 = t0 // 128
                                eng = dma_eng if dma_eng is not None \
                                    else nc.gpsimd
                                eng.dma_start(
                                    out=out_r[:, tb0:tb0 + 4,
                                              og * 512:(og + 1) * 512],
                                    in_=ot_sb)
                        pending.append(filler)

        # drain remaining tails, then the last tile's output-projection
        # groups; route half their DMAs to the now-idle Sync HWDGE queue
        emit_deferred()
        di = 0
        while pending:
            pending.popleft()(nc.sync if di % 2 == 0 else None)
            di += 1


_CACHE = {}


def _get_nc(plan_key, plan, n_mtiles):
    if plan_key not in _CACHE:
        _CACHE[plan_key] = _build(plan, n_mtiles)
    return _CACHE[plan_key]


def _prep_inputs(x, freqs_cis, mask, wq, wk, wv, wo):
    x = np.asarray(x, np.float32).reshape(T, D)
    wq = np.asarray(wq, np.float32)
    wk = np.asarray(wk, np.float32)
    wv = np.asarray(wv, np.float32)
    wo = np.asarray(wo, np.float32)
    fc = np.asarray(freqs_cis, np.float32)

    xt = np.ascontiguousarray(x.T).astype(BF16)            # (D, T)
    permq = _pair_split_perm(N_HEADS)
    permk = _pair_split_perm(N_KV)
    wq_p = wq[permq]
    wk_p = wk[permk]

    cos = fc[:, :, 0].T                                    # (64, S)
    sin = fc[:, :, 1].T
    csa = np.concatenate([cos, cos], axis=0).astype(BF16)  # (128, S)
    csb = np.concatenate([-sin, sin], axis=0).astype(BF16)

    plan, mtiles = _classify_mask(mask)

    in_maps = []
    for c in range(NCORES):
        wqt_c = np.ascontiguousarray(wq_p[c * E:(c + 1) * E].T).astype(BF16)
        wkt_c = np.ascontiguousarray(wk_p[c * HD:(c + 1) * HD].T).astype(BF16)
        wvt_c = np.ascontiguousarray(wv[c * HD:(c + 1) * HD].T).astype(BF16)
        wol_c = np.ascontiguousarray(wo[:, c * E:(c + 1) * E].T).astype(BF16)
        in_maps.append({
            "xt": xt, "wqt": wqt_c, "wkt": wkt_c, "wvt": wvt_c,
            "wol": wol_c, "csa": csa, "csb": csb, "mtl": mtiles,
        })
    return in_maps, plan, mtiles


def kernel(x, freqs_cis, mask, wq, wk, wv, wo, start_pos=0, **_unused):
    in_maps, plan, mtiles = _prep_inputs(x, freqs_cis, mask, wq, wk, wv, wo)
    plan_key = (tuple(tuple(r) for r in plan), mtiles.shape[0])
    nc = _get_nc(plan_key, plan, mtiles.shape[0])
    last_err = None
    for _attempt in range(3):
        try:
            res = run_bass_kernel_spmd(nc, in_maps,
                                       core_ids=list(range(NCORES)))
            break
        except Exception as e:  # transient NRT device errors — retry
            last_err = e
    else:
        raise last_err
    full = np.zeros((T, D), np.float32)
    for c in range(NCORES):
        full += np.asarray(res.results[c]["out"], np.float32)
    return full.reshape(B, S, D).astype(np.float32)
